# revision 6
# baseline (speedup 1.0000x reference)
"""Trainium2 Bass kernel for conditional-adjustment conv (CAConv).

Per sample b: h = relu(c[b] @ mlp_w1 + mlp_b1); adj = h @ mlp_w2 + mlp_b2;
w[b] = conv_w + adj.reshape(Co,Ci,3,3); out[b] = conv2d(x[b], w[b], pad=1) + conv_b.

Sharding: data-parallel over batch, 4 samples per core on 8 cores (SPMD).

Per-core device kernel (all conv/MLP matmuls in float32r — full-rate fp32):
  Stage A (weight gen): one small matmul + relu produces hT[17,4] (the 17th
  row is ones, folded in on the host via an extra ones row in c/w1, so the
  mlp_b2 row of w2p applies). adj[b, t, ci, co] = hT.T @ w2p is computed in
  9 tap-chunks; each sample's [ci, co] block is DMA-scattered into a
  per-pair tile wadj[ci + 64*half, t*64 + co]. Per-pair block-diagonal
  weights wblk[ci + 64*half, t*128 + 64*half + co] start as a host-built
  conv_w image (zeros off-diagonal) and get wadj added on the diagonal.
  Stage B (conv): host-padded x (130x130) lives as [ci(2 samples), h, w]
  on the 128 partitions. Each output chunk po[128, 512] (2 samples x 64 co
  partitions, 4 h-rows x 128 w free) accumulates 9 shift-tap matmuls with
  K=128 block-diagonal lhsT — one matmul per tap covers both samples.
"""

import sys

if "/opt/trn_rl_repo" not in sys.path:
    sys.path.insert(0, "/opt/trn_rl_repo")

import numpy as np

B = 32
NCORES = 8
BPC = B // NCORES          # samples per core = 4
PAIRS = BPC // 2           # sample pairs per core = 2
CIN = COUT = 64
H = W = 128
HP = WP = 130              # padded dims
KH = KW = 3
NT = KH * KW               # taps = 9
CL = 8                     # c length
CL1 = CL + 1               # + ones row
MH = 16                    # mlp hidden
K2 = MH + 1                # mlp hidden + ones row for mlp_b2
NCH = (H * W) // 512       # 512-col output chunks per pair = 32

_CACHE = {}


def _build():
    import concourse.bass as bass
    import concourse.mybir as mybir
    import concourse.tile as tile
    from concourse import bacc

    f32 = mybir.dt.float32
    f32r = mybir.dt.float32r
    AF = mybir.ActivationFunctionType

    nc = bacc.Bacc("TRN2", target_bir_lowering=False, debug=False)

    xs_d = nc.dram_tensor("xsp", [BPC, CIN, HP * WP], f32r, kind="ExternalInput")
    ct_d = nc.dram_tensor("cT", [CL1, BPC], f32, kind="ExternalInput")
    w1_d = nc.dram_tensor("w1", [CL1, K2], f32, kind="ExternalInput")
    b1_d = nc.dram_tensor("b1", [K2, 1], f32, kind="ExternalInput")
    w2_d = nc.dram_tensor("w2p", [K2, NT * CIN * COUT], f32r, kind="ExternalInput")
    cw_d = nc.dram_tensor("cwd", [128, NT * 128], f32r, kind="ExternalInput")
    cb_d = nc.dram_tensor("cb2", [128, 1], f32, kind="ExternalInput")
    out_d = nc.dram_tensor("out", [BPC, COUT, H, W], f32, kind="ExternalOutput")

    with tile.TileContext(nc) as tc:
        with (
            tc.tile_pool(name="consts", bufs=1) as consts,
            tc.tile_pool(name="w2pool", bufs=1) as w2pool,
            tc.tile_pool(name="adjpool", bufs=1) as adjpool,
            tc.tile_pool(name="xpool", bufs=2) as xpool,
            tc.tile_pool(name="opool", bufs=4) as opool,
            tc.tile_pool(name="pspool", bufs=8, space=bass.MemorySpace.PSUM) as ps,
        ):
            # ---- constants in ----
            ct_sb = consts.tile([CL1, BPC], f32)
            nc.sync.dma_start(out=ct_sb[:], in_=ct_d.ap())
            w1_sb = consts.tile([CL1, K2], f32)
            nc.sync.dma_start(out=w1_sb[:], in_=w1_d.ap())
            b1_sb = consts.tile([K2, 1], f32)
            nc.sync.dma_start(out=b1_sb[:], in_=b1_d.ap())
            cb_sb = consts.tile([128, 1], f32)
            nc.sync.dma_start(out=cb_sb[:], in_=cb_d.ap())

            # ---- stage A: conditioning MLP ----
            # hT[m, b] = relu(sum_k w1'[k, m] cT'[k, b] + b1'[m]); row MH is
            # relu(1) = 1 via the host-appended ones rows.
            ph = ps.tile([K2, BPC], f32, tag="ps")
            nc.tensor.matmul(ph[:], w1_sb[:], ct_sb[:], start=True, stop=True)
            ht_sb = consts.tile([K2, BPC], f32r)
            nc.scalar.activation(out=ht_sb[:], in_=ph[:], func=AF.Relu, bias=b1_sb[:])

            # per-pair adj scatter target: wadj[ci + 64*half, t*64 + co]
            wadj = [
                consts.tile([128, NT * COUT], f32r, name=f"wadj{p}", tag=f"wadj{p}")
                for p in range(PAIRS)
            ]
            # per-pair block-diag weights: wblk[ci+64*half, t*128 + 64*half + co]
            wblk = [
                consts.tile([128, NT * 128], f32r, name=f"wblk{p}", tag=f"wblk{p}")
                for p in range(PAIRS)
            ]
            for p in range(PAIRS):
                nc.sync.dma_start(out=wblk[p][:], in_=cw_d.ap())

            # adj[b, t, ci, co] = sum_k hT[k, b] w2p[k, t, ci, co]
            for t in range(NT):
                w2t = w2pool.tile([K2, CIN * COUT], f32r)
                nc.sync.dma_start(
                    out=w2t[:], in_=w2_d.ap()[:, t * CIN * COUT : (t + 1) * CIN * COUT]
                )
                adj = adjpool.tile([BPC, CIN * COUT], f32r)
                for n in range(8):
                    pa = ps.tile([BPC, 512], f32, tag="ps")
                    nc.tensor.matmul(
                        pa[:],
                        ht_sb[:],
                        w2t[:, n * 512 : (n + 1) * 512],
                        start=True,
                        stop=True,
                    )
                    nc.any.tensor_copy(adj[:, n * 512 : (n + 1) * 512], pa[:])
                # scatter each sample's [ci, co] block into its pair tile
                for b in range(BPC):
                    p, half = divmod(b, 2)
                    nc.sync.dma_start(
                        out=wadj[p][half * 64 : half * 64 + 64, t * 64 : t * 64 + 64],
                        in_=adj[b : b + 1, :],
                    )
            # add adj onto the diagonal blocks of wblk
            for p in range(PAIRS):
                for t in range(NT):
                    for half in range(2):
                        q = half * 64
                        dst = wblk[p][q : q + 64, t * 128 + q : t * 128 + q + 64]
                        nc.vector.tensor_add(
                            dst, dst, wadj[p][q : q + 64, t * 64 : t * 64 + 64]
                        )

            # ---- stage B: per-pair conv ----
            for p in range(PAIRS):
                xp = xpool.tile([128, HP * WP], f32r)
                xp3 = xp.rearrange("p (h w) -> p h w", w=WP)
                for half in range(2):
                    b = 2 * p + half
                    nc.sync.dma_start(
                        out=xp[half * 64 : half * 64 + 64, :], in_=xs_d.ap()[b]
                    )
                for ch in range(NCH):
                    h0 = ch * 4
                    po = ps.tile([128, 512], f32, tag="ps")
                    for t in range(NT):
                        kh, kw = divmod(t, 3)
                        nc.tensor.matmul(
                            po[:],
                            wblk[p][:, t * 128 : (t + 1) * 128],
                            xp3[:, h0 + kh : h0 + kh + 4, kw : kw + W],
                            start=(t == 0),
                            stop=(t == NT - 1),
                        )
                    os = opool.tile([128, 512], f32)
                    nc.vector.tensor_scalar_add(os[:], po[:], cb_sb[:])
                    nc.sync.dma_start(
                        out=out_d.ap()[2 * p : 2 * p + 2, :, h0 : h0 + 4, :],
                        in_=os[:],
                    )

    nc.compile()
    return nc


def _get_nc():
    if "nc" not in _CACHE:
        _CACHE["nc"] = _build()
    return _CACHE["nc"]


def _prep(x, c, conv_w, conv_b, mlp_w1, mlp_b1, mlp_w2, mlp_b2):
    x = np.ascontiguousarray(x, dtype=np.float32)
    c = np.ascontiguousarray(c, dtype=np.float32)
    conv_w = np.asarray(conv_w, dtype=np.float32)
    conv_b = np.asarray(conv_b, dtype=np.float32)
    mlp_w1 = np.asarray(mlp_w1, dtype=np.float32)
    mlp_b1 = np.asarray(mlp_b1, dtype=np.float32)
    mlp_w2 = np.asarray(mlp_w2, dtype=np.float32)
    mlp_b2 = np.asarray(mlp_b2, dtype=np.float32)

    # padded x, flattened spatial
    xsp = np.zeros((B, CIN, HP, WP), dtype=np.float32)
    xsp[:, :, 1 : HP - 1, 1 : WP - 1] = x
    xsp = xsp.reshape(B, CIN, HP * WP)

    # w1' [CL1, K2]: [[w1, 0], [0, 1]]; cT' [CL1, BPC] with ones row
    w19 = np.zeros((CL1, K2), dtype=np.float32)
    w19[:CL, :MH] = mlp_w1
    w19[CL, MH] = 1.0
    b117 = np.concatenate([mlp_b1, np.zeros(1, np.float32)]).reshape(K2, 1)
    b117 = np.ascontiguousarray(b117, dtype=np.float32)

    # w2p[k, t, ci, co] = mlp_w2[k, co*576 + ci*9 + t]; extra row = mlp_b2
    w2p = mlp_w2.reshape(MH, COUT, CIN, NT).transpose(0, 3, 2, 1)
    b2p = mlp_b2.reshape(COUT, CIN, NT).transpose(2, 1, 0)
    w2p = np.concatenate(
        [w2p.reshape(MH, -1), b2p.reshape(1, -1)], axis=0
    ).astype(np.float32)
    w2p = np.ascontiguousarray(w2p)

    # cwd[ci + 64*half, t*128 + 64*half + co] = conv_w[co, ci, t]; 0 off-diag
    cwp = conv_w.reshape(COUT, CIN, NT).transpose(1, 2, 0)  # [ci, t, co]
    cwd = np.zeros((128, NT, 128), dtype=np.float32)
    cwd[:64, :, :64] = cwp.transpose(0, 1, 2)
    cwd[64:, :, 64:] = cwp
    cwd = np.ascontiguousarray(cwd.reshape(128, NT * 128))
    cb2 = np.ascontiguousarray(
        np.tile(conv_b.reshape(COUT, 1), (2, 1)), dtype=np.float32
    )

    in_maps = []
    for i in range(NCORES):
        sl = slice(i * BPC, (i + 1) * BPC)
        ct9 = np.concatenate(
            [c[sl].T, np.ones((1, BPC), np.float32)], axis=0
        )
        in_maps.append(
            {
                "xsp": np.ascontiguousarray(xsp[sl]),
                "cT": np.ascontiguousarray(ct9),
                "w1": w19,
                "b1": b117,
                "w2p": w2p,
                "cwd": cwd,
                "cb2": cb2,
            }
        )
    return in_maps


def _run(inputs, trace=False):
    from concourse.bass_utils import run_bass_kernel_spmd

    nc = _get_nc()
    in_maps = _prep(**inputs)
    res = run_bass_kernel_spmd(
        nc, in_maps, core_ids=list(range(NCORES)), trace=trace
    )
    out = np.concatenate([res.results[i]["out"] for i in range(NCORES)], axis=0)
    return out, res


def kernel(**inputs):
    out, _ = _run(inputs, trace=False)
    return out


# revision 7
# speedup vs baseline: 1.0880x; 1.0880x over previous
"""Trainium2 Bass kernel for conditional-adjustment conv (CAConv).

Per sample b: h = relu(c[b] @ mlp_w1 + mlp_b1); adj = h @ mlp_w2 + mlp_b2;
w[b] = conv_w + adj.reshape(Co,Ci,3,3); out[b] = conv2d(x[b], w[b], pad=1) + conv_b.

Sharding: data-parallel over batch, 4 samples per core on 8 cores (SPMD).

Per-core device kernel (all conv/MLP matmuls in float32r — full-rate fp32):
  Stage A (weight gen): one small matmul + relu produces hT[17,4] (the 17th
  row is ones, folded in on the host via an extra ones row in c/w1, so the
  mlp_b2 row of w2p applies). adj[b, t, ci, co] = hT.T @ w2p is computed in
  9 tap-chunks; each sample's [ci, co] block is DMA-scattered into a
  per-pair tile wadj[ci + 64*half, t*64 + co]. Per-pair block-diagonal
  weights wblk[ci + 64*half, t*128 + 64*half + co] start as a host-built
  conv_w image (zeros off-diagonal) and get wadj added on the diagonal.
  Stage B (conv): host-padded x (130x130) lives as [ci(2 samples), h, w]
  on the 128 partitions. Each output chunk po[128, 512] (2 samples x 64 co
  partitions, 4 h-rows x 128 w free) accumulates 9 shift-tap matmuls with
  K=128 block-diagonal lhsT — one matmul per tap covers both samples.
"""

import sys

if "/opt/trn_rl_repo" not in sys.path:
    sys.path.insert(0, "/opt/trn_rl_repo")

import numpy as np

B = 32
NCORES = 8
BPC = B // NCORES          # samples per core = 4
PAIRS = BPC // 2           # sample pairs per core = 2
CIN = COUT = 64
H = W = 128
HP = WP = 130              # padded dims
KH = KW = 3
NT = KH * KW               # taps = 9
CL = 8                     # c length
CL1 = CL + 1               # + ones row
MH = 16                    # mlp hidden
K2 = MH + 1                # mlp hidden + ones row for mlp_b2
NCH = (H * W) // 512       # 512-col output chunks per pair = 32

_CACHE = {}


def _build():
    import concourse.bass as bass
    import concourse.mybir as mybir
    import concourse.tile as tile
    from concourse import bacc

    f32 = mybir.dt.float32
    f32r = mybir.dt.float32r
    AF = mybir.ActivationFunctionType

    nc = bacc.Bacc("TRN2", target_bir_lowering=False, debug=False)

    xs_d = nc.dram_tensor("xsp", [BPC, CIN, HP * WP], f32r, kind="ExternalInput")
    ct_d = nc.dram_tensor("cT", [CL1, BPC], f32, kind="ExternalInput")
    w1_d = nc.dram_tensor("w1", [CL1, K2], f32, kind="ExternalInput")
    b1_d = nc.dram_tensor("b1", [K2, 1], f32, kind="ExternalInput")
    w2_d = nc.dram_tensor("w2p", [K2, NT * CIN * COUT], f32r, kind="ExternalInput")
    cw_d = nc.dram_tensor("cwd", [128, NT * 128], f32r, kind="ExternalInput")
    cb_d = nc.dram_tensor("cb2", [128, 1], f32, kind="ExternalInput")
    out_d = nc.dram_tensor("out", [BPC, COUT, H, W], f32, kind="ExternalOutput")

    with tile.TileContext(nc) as tc:
        with (
            tc.tile_pool(name="consts", bufs=1) as consts,
            tc.tile_pool(name="w2pool", bufs=2) as w2pool,
            tc.tile_pool(name="adjpool", bufs=2) as adjpool,
            tc.tile_pool(name="xpool", bufs=2) as xpool,
            tc.tile_pool(name="opool", bufs=4) as opool,
            tc.tile_pool(name="pspool", bufs=8, space=bass.MemorySpace.PSUM) as ps,
        ):
            # ---- constants in ----
            ct_sb = consts.tile([CL1, BPC], f32)
            nc.sync.dma_start(out=ct_sb[:], in_=ct_d.ap())
            w1_sb = consts.tile([CL1, K2], f32)
            nc.sync.dma_start(out=w1_sb[:], in_=w1_d.ap())
            b1_sb = consts.tile([K2, 1], f32)
            nc.sync.dma_start(out=b1_sb[:], in_=b1_d.ap())
            cb_sb = consts.tile([128, 1], f32)
            nc.sync.dma_start(out=cb_sb[:], in_=cb_d.ap())

            # ---- stage A: conditioning MLP ----
            # hT[m, b] = relu(sum_k w1'[k, m] cT'[k, b] + b1'[m]); row MH is
            # relu(1) = 1 via the host-appended ones rows.
            ph = ps.tile([K2, BPC], f32, tag="ps")
            nc.tensor.matmul(ph[:], w1_sb[:], ct_sb[:], start=True, stop=True)
            ht_sb = consts.tile([K2, BPC], f32r)
            nc.scalar.activation(out=ht_sb[:], in_=ph[:], func=AF.Relu, bias=b1_sb[:])

            # per-pair adj scatter target: wadj[ci + 64*half, t*64 + co]
            wadj = [
                consts.tile([128, NT * COUT], f32r, name=f"wadj{p}", tag=f"wadj{p}")
                for p in range(PAIRS)
            ]
            # per-pair block-diag weights: wblk[ci+64*half, t*128 + 64*half + co]
            wblk = [
                consts.tile([128, NT * 128], f32r, name=f"wblk{p}", tag=f"wblk{p}")
                for p in range(PAIRS)
            ]
            for p in range(PAIRS):
                nc.sync.dma_start(out=wblk[p][:], in_=cw_d.ap())

            # adj[b, t, ci, co] = sum_k hT[k, b] w2p[k, t, ci, co]
            for t in range(NT):
                adj = adjpool.tile([BPC, CIN * COUT], f32r)
                for hf in range(2):
                    w2t = w2pool.tile([K2, 2048], f32r)
                    nc.sync.dma_start(
                        out=w2t[:],
                        in_=w2_d.ap()[
                            :, t * CIN * COUT + hf * 2048 : t * CIN * COUT + (hf + 1) * 2048
                        ],
                    )
                    for n in range(4):
                        pa = ps.tile([BPC, 512], f32, tag="ps")
                        nc.tensor.matmul(
                            pa[:],
                            ht_sb[:],
                            w2t[:, n * 512 : (n + 1) * 512],
                            start=True,
                            stop=True,
                        )
                        nc.any.tensor_copy(
                            adj[:, hf * 2048 + n * 512 : hf * 2048 + (n + 1) * 512],
                            pa[:],
                        )
                # scatter each sample's [ci, co] block into its pair tile
                for b in range(BPC):
                    p, half = divmod(b, 2)
                    nc.sync.dma_start(
                        out=wadj[p][half * 64 : half * 64 + 64, t * 64 : t * 64 + 64],
                        in_=adj[b : b + 1, :],
                    )
            # add adj onto the diagonal blocks of wblk
            for p in range(PAIRS):
                for t in range(NT):
                    for half in range(2):
                        q = half * 64
                        dst = wblk[p][q : q + 64, t * 128 + q : t * 128 + q + 64]
                        nc.vector.tensor_add(
                            dst, dst, wadj[p][q : q + 64, t * 64 : t * 64 + 64]
                        )

            # ---- stage B: per-pair conv ----
            for p in range(PAIRS):
                xp = xpool.tile([128, HP * WP], f32r)
                xp3 = xp.rearrange("p (h w) -> p h w", w=WP)
                for half in range(2):
                    b = 2 * p + half
                    nc.scalar.dma_start(
                        out=xp[half * 64 : half * 64 + 64, :], in_=xs_d.ap()[b]
                    )
                for ch in range(NCH):
                    h0 = ch * 4
                    po = ps.tile([128, 512], f32, tag="ps")
                    for t in range(NT):
                        kh, kw = divmod(t, 3)
                        nc.tensor.matmul(
                            po[:],
                            wblk[p][:, t * 128 : (t + 1) * 128],
                            xp3[:, h0 + kh : h0 + kh + 4, kw : kw + W],
                            start=(t == 0),
                            stop=(t == NT - 1),
                        )
                    os = opool.tile([128, 512], f32)
                    nc.vector.tensor_scalar_add(os[:], po[:], cb_sb[:])
                    nc.sync.dma_start(
                        out=out_d.ap()[2 * p : 2 * p + 2, :, h0 : h0 + 4, :],
                        in_=os[:],
                    )

    nc.compile()
    return nc


def _get_nc():
    if "nc" not in _CACHE:
        _CACHE["nc"] = _build()
    return _CACHE["nc"]


def _prep(x, c, conv_w, conv_b, mlp_w1, mlp_b1, mlp_w2, mlp_b2):
    x = np.ascontiguousarray(x, dtype=np.float32)
    c = np.ascontiguousarray(c, dtype=np.float32)
    conv_w = np.asarray(conv_w, dtype=np.float32)
    conv_b = np.asarray(conv_b, dtype=np.float32)
    mlp_w1 = np.asarray(mlp_w1, dtype=np.float32)
    mlp_b1 = np.asarray(mlp_b1, dtype=np.float32)
    mlp_w2 = np.asarray(mlp_w2, dtype=np.float32)
    mlp_b2 = np.asarray(mlp_b2, dtype=np.float32)

    # padded x, flattened spatial
    xsp = np.zeros((B, CIN, HP, WP), dtype=np.float32)
    xsp[:, :, 1 : HP - 1, 1 : WP - 1] = x
    xsp = xsp.reshape(B, CIN, HP * WP)

    # w1' [CL1, K2]: [[w1, 0], [0, 1]]; cT' [CL1, BPC] with ones row
    w19 = np.zeros((CL1, K2), dtype=np.float32)
    w19[:CL, :MH] = mlp_w1
    w19[CL, MH] = 1.0
    b117 = np.concatenate([mlp_b1, np.zeros(1, np.float32)]).reshape(K2, 1)
    b117 = np.ascontiguousarray(b117, dtype=np.float32)

    # w2p[k, t, ci, co] = mlp_w2[k, co*576 + ci*9 + t]; extra row = mlp_b2
    w2p = mlp_w2.reshape(MH, COUT, CIN, NT).transpose(0, 3, 2, 1)
    b2p = mlp_b2.reshape(COUT, CIN, NT).transpose(2, 1, 0)
    w2p = np.concatenate(
        [w2p.reshape(MH, -1), b2p.reshape(1, -1)], axis=0
    ).astype(np.float32)
    w2p = np.ascontiguousarray(w2p)

    # cwd[ci + 64*half, t*128 + 64*half + co] = conv_w[co, ci, t]; 0 off-diag
    cwp = conv_w.reshape(COUT, CIN, NT).transpose(1, 2, 0)  # [ci, t, co]
    cwd = np.zeros((128, NT, 128), dtype=np.float32)
    cwd[:64, :, :64] = cwp.transpose(0, 1, 2)
    cwd[64:, :, 64:] = cwp
    cwd = np.ascontiguousarray(cwd.reshape(128, NT * 128))
    cb2 = np.ascontiguousarray(
        np.tile(conv_b.reshape(COUT, 1), (2, 1)), dtype=np.float32
    )

    in_maps = []
    for i in range(NCORES):
        sl = slice(i * BPC, (i + 1) * BPC)
        ct9 = np.concatenate(
            [c[sl].T, np.ones((1, BPC), np.float32)], axis=0
        )
        in_maps.append(
            {
                "xsp": np.ascontiguousarray(xsp[sl]),
                "cT": np.ascontiguousarray(ct9),
                "w1": w19,
                "b1": b117,
                "w2p": w2p,
                "cwd": cwd,
                "cb2": cb2,
            }
        )
    return in_maps


def _run(inputs, trace=False):
    from concourse.bass_utils import run_bass_kernel_spmd

    nc = _get_nc()
    in_maps = _prep(**inputs)
    res = run_bass_kernel_spmd(
        nc, in_maps, core_ids=list(range(NCORES)), trace=trace
    )
    out = np.concatenate([res.results[i]["out"] for i in range(NCORES)], axis=0)
    return out, res


def kernel(**inputs):
    out, _ = _run(inputs, trace=False)
    return out


# revision 8
# speedup vs baseline: 1.2338x; 1.1340x over previous
"""Trainium2 Bass kernel for conditional-adjustment conv (CAConv).

Per sample b: h = relu(c[b] @ mlp_w1 + mlp_b1); adj = h @ mlp_w2 + mlp_b2;
w[b] = conv_w + adj.reshape(Co,Ci,3,3); out[b] = conv2d(x[b], w[b], pad=1) + conv_b.

Sharding: data-parallel over batch, 4 samples per core on 8 cores (SPMD).

Per-core device kernel (all big matmuls in float32r — full-rate fp32 on PE):
  Stage A (weight gen): one small matmul + relu produces hT[17,4]; the 17th
  row is ones (host-appended ones rows in c/w1), so row 16 of w2p — which
  the host sets to mlp_b2 + conv_w, both permuted — rides along and the
  scattered result is directly the complete per-sample conv weight.
  adj[b, t, ci, co] = hT.T @ w2p in 9 tap-chunks; each sample's [ci, co]
  block is DMA-scattered (SWDGE) into the diagonal blocks of the per-pair
  block-diagonal weight tile wblk[ci + 64*half, t*128 + 64*half + co],
  whose off-diagonal zeros come from a broadcast-DMA of a zeros vector.
  Stage B (conv): host-padded x (130x130) for a sample pair lives as
  [ci(2 samples), h, w] across the 128 partitions. Each output chunk
  po[128, 512] (2 samples x 64 co partitions; 4 h-rows x 128 w free)
  accumulates 9 shift-tap K=128 matmuls — one matmul per tap covers both
  samples. Bias is added during the PSUM->SBUF copy, then DMA to DRAM.

  DMA queues: sync HWDGE = consts + w2 chunks + output stores (small or
  late - keeps stage-A loads low-latency); ACT HWDGE = the two bulk x
  loads (128-partition, descriptor-size-capped so queue round-robin stays
  fair); GPSIMD SWDGE = weight scatters + zero-fills (latency-tolerant).
"""

import sys

if "/opt/trn_rl_repo" not in sys.path:
    sys.path.insert(0, "/opt/trn_rl_repo")

import numpy as np

B = 32
NCORES = 8
BPC = B // NCORES          # samples per core = 4
PAIRS = BPC // 2           # sample pairs per core = 2
CIN = COUT = 64
H = W = 128
HP = WP = 130              # padded dims
KH = KW = 3
NT = KH * KW               # taps = 9
CL = 8                     # c length
CL1 = CL + 1               # + ones row
MH = 16                    # mlp hidden
K2 = MH + 1                # mlp hidden + ones row
NCH = (H * W) // 512       # 512-col output chunks per pair = 32

_CACHE = {}


def _build():
    import concourse.bass as bass
    import concourse.mybir as mybir
    import concourse.tile as tile
    from concourse import bacc

    f32 = mybir.dt.float32
    f32r = mybir.dt.float32r
    AF = mybir.ActivationFunctionType

    nc = bacc.Bacc("TRN2", target_bir_lowering=False, debug=False)

    xs_d = nc.dram_tensor("xsp", [BPC, CIN, HP * WP], f32r, kind="ExternalInput")
    ct_d = nc.dram_tensor("cT", [CL1, BPC], f32, kind="ExternalInput")
    w1_d = nc.dram_tensor("w1", [CL1, K2], f32, kind="ExternalInput")
    b1_d = nc.dram_tensor("b1", [K2, 1], f32, kind="ExternalInput")
    w2_d = nc.dram_tensor("w2p", [K2, NT * CIN * COUT], f32r, kind="ExternalInput")
    zz_d = nc.dram_tensor("zz", [1, NT * 128], f32r, kind="ExternalInput")
    cb_d = nc.dram_tensor("cb2", [128, 1], f32, kind="ExternalInput")
    out_d = nc.dram_tensor("out", [BPC, COUT, H, W], f32, kind="ExternalOutput")

    with tile.TileContext(nc) as tc:
        with (
            tc.tile_pool(name="consts", bufs=1) as consts,
            tc.tile_pool(name="w2pool", bufs=2) as w2pool,
            tc.tile_pool(name="adjpool", bufs=2) as adjpool,
            tc.tile_pool(name="xpool", bufs=2) as xpool,
            tc.tile_pool(name="opool", bufs=4) as opool,
            tc.tile_pool(name="pspool", bufs=8, space=bass.MemorySpace.PSUM) as ps,
        ):
            # ---- constants in (sync queue; kept small + early) ----
            ct_sb = consts.tile([CL1, BPC], f32)
            nc.sync.dma_start(out=ct_sb[:], in_=ct_d.ap())
            w1_sb = consts.tile([CL1, K2], f32)
            nc.sync.dma_start(out=w1_sb[:], in_=w1_d.ap())
            b1_sb = consts.tile([K2, 1], f32)
            nc.sync.dma_start(out=b1_sb[:], in_=b1_d.ap())
            cb_sb = consts.tile([128, 1], f32)
            nc.sync.dma_start(out=cb_sb[:], in_=cb_d.ap())

            # ---- bulk x loads: one 128-partition DMA per pair (ACT queue),
            # descriptor size capped so other queues stay responsive ----
            xps = []
            for p in range(PAIRS):
                xp = xpool.tile([128, HP * WP], f32r, name=f"xp{p}", tag="xp")
                nc.scalar.dma_start(
                    out=xp[:],
                    in_=xs_d.ap()[2 * p : 2 * p + 2],
                    max_dma_last_dim=1690,
                )
                xps.append(xp)

            # per-pair block-diag weights; off-diag zero-filled via
            # broadcast DMA (SWDGE)
            wblk = []
            for p in range(PAIRS):
                wb = consts.tile([128, NT * 128], f32r, name=f"wblk{p}", tag=f"wblk{p}")
                zsrc = bass.AP(
                    tensor=zz_d.ap().tensor, offset=0, ap=[[0, 128], [1, NT * 128]]
                )
                nc.gpsimd.dma_start(out=wb[:], in_=zsrc)
                wblk.append(wb)

            # ---- stage A: conditioning MLP ----
            ph = ps.tile([K2, BPC], f32, tag="ps")
            nc.tensor.matmul(ph[:], w1_sb[:], ct_sb[:], start=True, stop=True)
            ht_sb = consts.tile([K2, BPC], f32r)
            nc.scalar.activation(out=ht_sb[:], in_=ph[:], func=AF.Relu, bias=b1_sb[:])

            # adj[b, t, ci, co] = sum_k hT[k, b] w2p[k, t, ci, co]
            # (w2p row 16 carries mlp_b2 + conv_w, so adj is the full weight)
            for t in range(NT):
                adj = adjpool.tile([BPC, CIN * COUT], f32r)
                for hf in range(2):
                    w2t = w2pool.tile([K2, 2048], f32r)
                    nc.sync.dma_start(
                        out=w2t[:],
                        in_=w2_d.ap()[
                            :, t * CIN * COUT + hf * 2048 : t * CIN * COUT + (hf + 1) * 2048
                        ],
                    )
                    for n in range(4):
                        pa = ps.tile([BPC, 512], f32, tag="ps")
                        nc.tensor.matmul(
                            pa[:],
                            ht_sb[:],
                            w2t[:, n * 512 : (n + 1) * 512],
                            start=True,
                            stop=True,
                        )
                        nc.any.tensor_copy(
                            adj[:, hf * 2048 + n * 512 : hf * 2048 + (n + 1) * 512],
                            pa[:],
                        )
                # scatter each sample's [ci, co] block onto wblk's diagonal
                for b in range(BPC):
                    p, half = divmod(b, 2)
                    q = half * 64
                    nc.gpsimd.dma_start(
                        out=wblk[p][q : q + 64, t * 128 + q : t * 128 + q + 64],
                        in_=adj[b : b + 1, :],
                    )

            # ---- stage B: per-pair conv ----
            for p in range(PAIRS):
                xp3 = xps[p].rearrange("p (h w) -> p h w", w=WP)
                for ch in range(NCH):
                    h0 = ch * 4
                    po = ps.tile([128, 512], f32, tag="ps")
                    for t in range(NT):
                        kh, kw = divmod(t, 3)
                        nc.tensor.matmul(
                            po[:],
                            wblk[p][:, t * 128 : (t + 1) * 128],
                            xp3[:, h0 + kh : h0 + kh + 4, kw : kw + W],
                            start=(t == 0),
                            stop=(t == NT - 1),
                        )
                    os = opool.tile([128, 512], f32)
                    nc.vector.tensor_scalar_add(os[:], po[:], cb_sb[:])
                    nc.sync.dma_start(
                        out=out_d.ap()[2 * p : 2 * p + 2, :, h0 : h0 + 4, :],
                        in_=os[:],
                    )

    nc.compile()
    return nc


def _get_nc():
    if "nc" not in _CACHE:
        _CACHE["nc"] = _build()
    return _CACHE["nc"]


def _prep(x, c, conv_w, conv_b, mlp_w1, mlp_b1, mlp_w2, mlp_b2):
    x = np.ascontiguousarray(x, dtype=np.float32)
    c = np.ascontiguousarray(c, dtype=np.float32)
    conv_w = np.asarray(conv_w, dtype=np.float32)
    conv_b = np.asarray(conv_b, dtype=np.float32)
    mlp_w1 = np.asarray(mlp_w1, dtype=np.float32)
    mlp_b1 = np.asarray(mlp_b1, dtype=np.float32)
    mlp_w2 = np.asarray(mlp_w2, dtype=np.float32)
    mlp_b2 = np.asarray(mlp_b2, dtype=np.float32)

    # padded x, flattened spatial
    xsp = np.zeros((B, CIN, HP, WP), dtype=np.float32)
    xsp[:, :, 1 : HP - 1, 1 : WP - 1] = x
    xsp = xsp.reshape(B, CIN, HP * WP)

    # w1' [CL1, K2]: [[w1, 0], [0, 1]]; cT' [CL1, BPC] gets a ones row
    w19 = np.zeros((CL1, K2), dtype=np.float32)
    w19[:CL, :MH] = mlp_w1
    w19[CL, MH] = 1.0
    b117 = np.concatenate([mlp_b1, np.zeros(1, np.float32)]).reshape(K2, 1)
    b117 = np.ascontiguousarray(b117, dtype=np.float32)

    # w2p[k, t, ci, co] = mlp_w2[k, co*576 + ci*9 + t]
    # row 16 = (mlp_b2 + conv_w), same permutation -> adj == full weight
    w2p = mlp_w2.reshape(MH, COUT, CIN, NT).transpose(0, 3, 2, 1)
    b2p = mlp_b2.reshape(COUT, CIN, NT).transpose(2, 1, 0)
    cwp = conv_w.reshape(COUT, CIN, NT).transpose(2, 1, 0)  # [t, ci, co]
    row16 = (b2p + cwp).reshape(1, -1)
    w2p = np.concatenate([w2p.reshape(MH, -1), row16], axis=0).astype(np.float32)
    w2p = np.ascontiguousarray(w2p)

    zz = np.zeros((1, NT * 128), dtype=np.float32)
    cb2 = np.ascontiguousarray(
        np.tile(conv_b.reshape(COUT, 1), (2, 1)), dtype=np.float32
    )

    in_maps = []
    for i in range(NCORES):
        sl = slice(i * BPC, (i + 1) * BPC)
        ct9 = np.concatenate([c[sl].T, np.ones((1, BPC), np.float32)], axis=0)
        in_maps.append(
            {
                "xsp": np.ascontiguousarray(xsp[sl]),
                "cT": np.ascontiguousarray(ct9),
                "w1": w19,
                "b1": b117,
                "w2p": w2p,
                "zz": zz,
                "cb2": cb2,
            }
        )
    return in_maps


def _run(inputs, trace=False):
    from concourse.bass_utils import run_bass_kernel_spmd

    nc = _get_nc()
    in_maps = _prep(**inputs)
    res = run_bass_kernel_spmd(
        nc, in_maps, core_ids=list(range(NCORES)), trace=trace
    )
    out = np.concatenate([res.results[i]["out"] for i in range(NCORES)], axis=0)
    return out, res


def kernel(**inputs):
    out, _ = _run(inputs, trace=False)
    return out
